# revision 5
# baseline (speedup 1.0000x reference)
"""Embedding lookup (gather) on 8 Trainium2 NeuronCores.

Strategy: data-parallel, bf16 table, int32 indirect gather, two
convert+store pipelines.

The [768, 50257] f32 table is transposed and converted to bf16 host-side
(the tolerance is rel_err < 2e-2; bf16 rounds at 2^-9 ~ 0.2% and, unlike
fp16, has no subnormal blow-up for near-zero weights) and replicated to
every core's DRAM as row-major [50257, 768] bf16. The 8*2048 = 16384 token
indices are sharded 2048 per core, 16 gather groups of 128 rows.

Rate analysis (measured on this part): SWDGE descriptor generation for
InstDMACopy is hardwired to Q7 cpu pair 0 and engine-serial at ~1.4us per
128-row indirect_dma_start, and a gather's DMAs only fire once its own
descgen finishes -> the 16 gathers issue over ~22.6us no matter what; that
chain is the kernel's spine. With an f32 table the issue pace put ~560 GB/s
of demand on a ~420 GB/s DMA fabric and everything slipped (47.7us). With
bf16 the gather stream halves (~3.15 MB at ~139 GB/s issue pace), stores
ride along at ~279 GB/s, and the kernel tracks the descgen spine + a short
tail. bf16 fabric traffic (9.45 MB ~ 22.5us) balances the descgen spine
almost exactly, so neither dominates; K-row-per-descriptor variants trade
descgen for fabric and land strictly worse on both sides of K=1.

Per group: SWDGE gather (bf16, 196KB) -> tensor_copy upconvert to f32 ->
HWDGE store (f32, 393KB). Even groups run on the DVE pipeline (convert on
nc.vector, store on SP's ring after the DVE EVSEM); odd groups run wholly
on ACT (convert then store in program order, no cross-engine sem), so the
last group's convert+store starts the moment its gather lands.

Raw Bass (no TileContext, no nc.Block): all-engine barriers cost ~3-4 us
each on a ~35 us kernel, so the init barrier + const memsets are stripped
from the module and engine streams are left unsynchronized except for the
semaphores that express real data dependencies:
  - SP loads the indices in two slices: group 0's 512B contiguous (its own
    tiny DRAM tensor, so Q7 can start generating gather 0's descriptors at
    the earliest possible moment), then the other 15 groups in one DMA that
    lands during gather 0's descgen.
  - GpSimd waits for the indices, then issues the 16 indirect gathers
    back-to-back (descgen-paced) on the single SWDGE ring; the ring
    carveout is tripled (dynamic_dma_scratch_size=49152) so descriptor
    reclaim never stalls descgen.
  - Gather i completes on its dedicated sem (gsems[i] >= 16; cumulative
    counts across SWDGE DMAs on one sem are unsound - the 16 increments
    per DMA come from 16 independently-progressing SDMA engines).
  - SP's final cumulative wait on ssem covers all stores before retire.
NOTE: the HW indirect DMA honors only the offset AP's partition dim
(<=128 indices per instruction) - a [128, 2] offset AP silently drops the
second column - so gathers are fixed at 128 rows each.
"""

import numpy as np

VOCAB = 50257
EMBED = 768
BATCH = 8
SEQ = 2048
N_CORES = 8
P = 128                      # SBUF partitions
TOK_PER_CORE = BATCH * SEQ // N_CORES   # 2048
GROUPS = TOK_PER_CORE // P              # 16 gather groups of 128 rows

_cached = {}
LAST_RESULTS = None  # BassKernelResults of the most recent run (for test harness)


def _build():
    """Build + compile the single-core Bass program (shared SPMD across 8 cores)."""
    import concourse.bacc as bacc
    import concourse.bass as bass
    from concourse import mybir

    nc = bacc.Bacc(
        "TRN2",
        target_bir_lowering=False,
        debug=False,
        num_devices=N_CORES,
        num_swdge_queues=1,
        dynamic_dma_scratch_size=49152,
    )

    # Drop the init-time const memsets and the all-engine barrier (~3.5 us):
    # nothing in this kernel reads the const APs, and the engine streams only
    # communicate through DMA semaphores which the loader zero-initializes.
    main_blk = nc.m.functions[0].blocks[0]
    removable = [
        inst
        for inst in main_blk.instructions
        if type(inst).__name__ in ("InstMemset", "InstDrain", "InstEventSemaphore")
    ]
    for inst in removable:
        main_blk.instructions.remove(inst)

    table = nc.dram_tensor(
        "table", [VOCAB, EMBED], mybir.dt.bfloat16, kind="ExternalInput"
    ).ap()
    # Group 0's indices as their own contiguous 512B tensor: the first idx
    # load is the head-latency critical path, and a [P, 1] tensor gives the
    # DMA one contiguous TX run instead of 128 strided 4B reads.
    idx0 = nc.dram_tensor("idx0", [P, 1], mybir.dt.int32, kind="ExternalInput").ap()
    idx = nc.dram_tensor(
        "idx", [P, GROUPS], mybir.dt.int32, kind="ExternalInput"
    ).ap()
    out = nc.dram_tensor(
        "out", [GROUPS, P, EMBED], mybir.dt.float32, kind="ExternalOutput"
    ).ap()

    import contextlib

    with contextlib.ExitStack() as ctx:
        idx_sb = ctx.enter_context(
            nc.sbuf_tensor("idx_sb", [P, GROUPS], mybir.dt.int32)
        )
        emb16 = ctx.enter_context(
            nc.sbuf_tensor("emb16", [P, GROUPS * EMBED], mybir.dt.bfloat16)
        )
        emb = ctx.enter_context(
            nc.sbuf_tensor("emb", [P, GROUPS * EMBED], mybir.dt.float32)
        )
        isem = ctx.enter_context(nc.semaphore("isem"))
        isem2 = ctx.enter_context(nc.semaphore("isem2"))
        vsem = ctx.enter_context(nc.semaphore("vsem"))
        ssem = ctx.enter_context(nc.semaphore("ssem"))
        gsems = [
            ctx.enter_context(nc.semaphore(f"gsem{i}")) for i in range(GROUPS)
        ]

        # SP: index loads (HWDGE - cheap descriptor gen, Q7 stays free).
        nc.sync.dma_start(idx_sb[:, :1], idx0[:]).then_inc(isem, 16)
        nc.sync.dma_start(idx_sb[:, 1:], idx[:, 1:]).then_inc(isem2, 16)

        # GpSimd/SWDGE: 16 indirect gathers, back-to-back (descgen-paced).
        nc.gpsimd.wait_ge(isem, 16)
        for i in range(GROUPS):
            if i == 1:
                nc.gpsimd.wait_ge(isem2, 16)
            nc.gpsimd.indirect_dma_start(
                out=emb16[:, i * EMBED : (i + 1) * EMBED],
                out_offset=None,
                in_=table[:],
                in_offset=bass.IndirectOffsetOnAxis(ap=idx_sb[:, i : i + 1], axis=0),
            ).then_inc(gsems[i], 16)

        # Even groups: DVE converts (bf16 -> f32), SP stores after the DVE
        # EVSEM. Odd groups: ACT converts then stores in program order - no
        # cross-engine sem, so the final group's tail is as short as possible.
        for g in range(0, GROUPS, 2):
            nc.vector.wait_ge(gsems[g], 16)
            nc.vector.tensor_copy(
                emb[:, g * EMBED : (g + 1) * EMBED],
                emb16[:, g * EMBED : (g + 1) * EMBED],
            ).then_inc(vsem, 1)
        for g in range(0, GROUPS, 2):
            nc.sync.wait_ge(vsem, g // 2 + 1)
            nc.sync.dma_start(out[g], emb[:, g * EMBED : (g + 1) * EMBED]).then_inc(
                ssem, 16
            )
        for g in range(1, GROUPS, 2):
            nc.scalar.wait_ge(gsems[g], 16)
            nc.scalar.copy(
                emb[:, g * EMBED : (g + 1) * EMBED],
                emb16[:, g * EMBED : (g + 1) * EMBED],
            )
            nc.scalar.dma_start(out[g], emb[:, g * EMBED : (g + 1) * EMBED]).then_inc(
                ssem, 16
            )

        # All stores landed (sem increments fire after last-byte receipt).
        # A cumulative wait is sound here: GROUPS*16 is the maximum total.
        nc.sync.wait_ge(ssem, GROUPS * 16)

    nc.compile()
    return nc


def _ensure_axon_hooks_importable():
    """bass_utils imports antenv.axon_hooks when BASS_TRACE is set under axon;
    the agent image's antenv package lacks that module. Provide a no-op shim
    so a stray BASS_TRACE env var cannot crash the run (tracing degrades)."""
    import sys
    import types

    try:
        import antenv.axon_hooks  # noqa: F401
        return
    except ImportError:
        pass
    try:
        import antenv
    except ImportError:
        return
    mod = types.ModuleType("antenv.axon_hooks")
    _h = [None]
    mod.set_axon_ntff_profile_hook = lambda h: _h.__setitem__(0, h)
    mod.get_axon_ntff_profile_hook = lambda: _h[0]
    sys.modules["antenv.axon_hooks"] = mod
    antenv.axon_hooks = mod


def kernel(x, weight):
    global LAST_RESULTS
    _ensure_axon_hooks_importable()
    from concourse.bass_utils import run_bass_kernel_spmd

    if "nc" not in _cached:
        _cached["nc"] = _build()
    nc = _cached["nc"]

    # Host-side input staging: transpose table to row-major [V, D] bf16;
    # shard tokens 2048/core, laid out [128 partitions, 16 groups] so group g
    # of core c covers tokens c*2048 + g*128 + p.
    import ml_dtypes

    wt = np.ascontiguousarray(
        np.asarray(weight, dtype=np.float32).T.astype(ml_dtypes.bfloat16)
    )
    x_flat = np.asarray(x, dtype=np.int32).reshape(N_CORES, TOK_PER_CORE)
    in_maps = []
    for c in range(N_CORES):
        idx_c = np.ascontiguousarray(x_flat[c].reshape(GROUPS, P).T)
        in_maps.append(
            {
                "table": wt,
                "idx0": np.ascontiguousarray(idx_c[:, :1]),
                "idx": idx_c,
            }
        )

    res = run_bass_kernel_spmd(nc, in_maps, core_ids=list(range(N_CORES)))
    LAST_RESULTS = res

    out = np.empty((N_CORES, TOK_PER_CORE, EMBED), dtype=np.float32)
    for c in range(N_CORES):
        out[c] = np.asarray(res.results[c]["out"]).reshape(TOK_PER_CORE, EMBED)
    return out.reshape(BATCH, SEQ, EMBED)
